# revision 26
# baseline (speedup 1.0000x reference)
"""MoE (top-2 of 8 experts, swiglu MLP) on 8 Trainium2 NeuronCores.

Strategy: expert parallelism — core e owns expert e's weights.
 - Host: router (fp64 softmax/top-2), gather each expert's tokens,
   pre-tile weights into the layouts the PE consumes directly, cast to
   bf16 (fp32 PSUM accumulation on device keeps the error ~3.5e-3 in
   the max-over-global-max metric).
 - Device (per core, SPMD one NEFF): single chunk of C=2048 columns
   (exactly T*K/8 — perfectly balanced), all matmul N-blocks 512.
   Stage A: hT = silu(gate_w.T @ xT) * (up_w.T @ xT); stage B:
   yT = down_w @ hT. Weights stream once (no re-chunking).
 - Host: combine — scale rows by gating weight and scatter-add into
   the full output. Tokens beyond the 2048-capacity of an expert are
   computed on the host (103 token-expert pairs for this input).

Shapes: T=8192 tokens, H=2048, F=1408, E=8, K=2, C=2048.
"""

import numpy as np

T, H, E, K, F = 8192, 2048, 8, 2, 1408
C = 2048  # token capacity per expert; overflow falls back to host
N_CORES = 8

_compiled = None


def _build():
    from contextlib import ExitStack

    import concourse.mybir as mybir
    import concourse.tile as tile
    from concourse import bacc

    f32 = mybir.dt.float32
    bf16 = mybir.dt.bfloat16

    nc = bacc.Bacc("TRN2", target_bir_lowering=False, debug=False, num_devices=N_CORES)
    xt = nc.dram_tensor("xt", [H, C], bf16, kind="ExternalInput").ap()
    gu = nc.dram_tensor("gu", [2, 11, 128, 2048], bf16, kind="ExternalInput").ap()
    dw = nc.dram_tensor("dw", [16, 128, 1408], bf16, kind="ExternalInput").ap()
    yt = nc.dram_tensor("yt", [H, C], bf16, kind="ExternalOutput").ap()

    with tile.TileContext(nc) as tc:
        with ExitStack() as ctx:
            pool_xt = ctx.enter_context(tc.tile_pool(name="xt", bufs=32))
            pool_gu = ctx.enter_context(tc.tile_pool(name="gu", bufs=3))
            pool_dw = ctx.enter_context(tc.tile_pool(name="dw", bufs=3))
            pool_h = ctx.enter_context(tc.tile_pool(name="h", bufs=11))
            pool_sil = ctx.enter_context(tc.tile_pool(name="sil", bufs=4))
            pool_out = ctx.enter_context(tc.tile_pool(name="out", bufs=4))
            pool_wrm = ctx.enter_context(tc.tile_pool(name="wrm", bufs=1))
            ps = ctx.enter_context(tc.tile_pool(name="ps", bufs=8, space="PSUM"))

            # PE warmup: the first ~16us are DMA-bound (startup loads), and
            # the PE's HAM clock gate only reaches 2.4 GHz after ~3.4us of
            # sustained matmul activity. Burn dummy matmuls on memset data
            # while the DMAs land so the real matmuls start at full clock.
            wrm_w = pool_wrm.tile([128, 128], bf16, tag="wrm_w")
            wrm_x = pool_wrm.tile([128, 512], bf16, tag="wrm_x")
            wrm_o = pool_wrm.tile([128, 512], f32, tag="wrm_o")
            nc.vector.memset(wrm_w[:], 0.0)
            nc.vector.memset(wrm_x[:], 0.0)
            wrm_p = ps.tile([128, 512], f32, tag="ps", name="wrm_p")
            N_WARM = 16
            for i in range(N_WARM):
                nc.tensor.matmul(
                    wrm_p[:], wrm_w[:], wrm_x[:],
                    start=(i == 0), stop=(i == N_WARM - 1),
                )
            nc.vector.tensor_copy(wrm_o[:], wrm_p[:])

            # f=0 weights lead the two HWDGE rings (they gate the first real
            # matmul); f=1,2 weights are queued behind the xt tiles below so
            # they don't steal HBM bandwidth from the delivery-bound startup
            # (they land ~37us, just before stage A reaches f=1 at ~41us)
            guts, uuts = {}, {}
            for f in range(3):
                guts[f] = pool_gu.tile([128, 2048], bf16, tag="gut", name="gut")
                uuts[f] = pool_gu.tile([128, 2048], bf16, tag="uut", name="uut")
            nc.sync.dma_start(guts[0][:], gu[0, 0])
            nc.scalar.dma_start(uuts[0][:], gu[1, 0])

            # token activations, H on partitions, as [128, 512] quarter-tiles
            # (one matmul rhs each) alternating across the two HWDGE rings in
            # exactly the order stage A consumes them. Fine granularity keeps
            # any late-tile PE stall well under the ~3.4us HAM idle window
            # that would re-throttle the PE clock to 1.2 GHz.
            xqs = [
                [[None, None] for _ in range(16)],
                [[None, None] for _ in range(16)],
            ]  # [half][hb][cb] -> [128, 512] tile
            for half in range(2):
                for hb in range(16):
                    eng = nc.sync if hb % 2 == 0 else nc.scalar
                    for cb in range(2):
                        xtile = pool_xt.tile([128, 512], bf16, tag="xtile", name="xt")
                        xqs[half][hb][cb] = xtile
                        c0 = half * 1024 + cb * 512
                        eng.dma_start(
                            xtile[:], xt[hb * 128 : (hb + 1) * 128, c0 : c0 + 512]
                        )
            for f in (1, 2):
                nc.sync.dma_start(guts[f][:], gu[0, f])
                nc.scalar.dma_start(uuts[f][:], gu[1, f])

            def silu_mul(pg, pu, ht, gcb):
                sil = pool_sil.tile([128, 512], f32, tag="sil", name="sil")
                nc.scalar.activation(sil[:], pg[:], mybir.ActivationFunctionType.Silu)
                nc.vector.tensor_mul(ht[:, gcb * 512 : (gcb + 1) * 512], sil[:], pu[:])

            # stage A: hT[f, c] = silu(gT) * uT, gT = gate_w.T @ x.T
            # g and u interleaved per h-tile in column-block pairs, so each
            # xt half-tile is fully consumed as soon as it lands.
            hts = []
            for f in range(11):
                if f in guts:
                    gut, uut = guts[f], uuts[f]
                else:
                    gut = pool_gu.tile([128, 2048], bf16, tag="gut", name="gut")
                    uut = pool_gu.tile([128, 2048], bf16, tag="uut", name="uut")
                    nc.sync.dma_start(gut[:], gu[0, f])
                    nc.scalar.dma_start(uut[:], gu[1, f])
                ht = pool_h.tile([128, C], bf16, tag="ht")
                hts.append(ht)
                for part in range(2):
                    cbs = (0, 1)  # column blocks within this half's tiles
                    pgs = [ps.tile([128, 512], f32, tag="ps", name="pg") for _ in cbs]
                    pus = [ps.tile([128, 512], f32, tag="ps", name="pu") for _ in cbs]
                    for h in range(16):
                        for cb, pg in zip(cbs, pgs):
                            nc.tensor.matmul(
                                pg[:],
                                gut[:, h * 128 : (h + 1) * 128],
                                xqs[part][h][cb][:],
                                start=(h == 0),
                                stop=(h == 15),
                            )
                        for cb, pu in zip(cbs, pus):
                            nc.tensor.matmul(
                                pu[:],
                                uut[:, h * 128 : (h + 1) * 128],
                                xqs[part][h][cb][:],
                                start=(h == 0),
                                stop=(h == 15),
                            )
                    for cb, pg, pu in zip(cbs, pgs, pus):
                        silu_mul(pg, pu, ht, 2 * part + cb)

            # stage B: yT[h, c] = down_w @ hT  (gating applied on host).
            # Output stored bf16 (upcast on host) — halves store traffic.
            for hb in range(16):
                dwt = pool_dw.tile([128, 1408], bf16, tag="dwt")
                nc.sync.dma_start(dwt[:], dw[hb])
                for cb in range(4):
                    if hb == 15 and cb == 3:
                        # final group: two 256-wide accumulations so the
                        # tail-gating cast+store chain covers 64KB, not 128KB
                        for q in range(2):
                            co = cb * 512 + q * 256
                            po = ps.tile([128, 256], f32, tag="ps", name="po")
                            for f in range(11):
                                nc.tensor.matmul(
                                    po[:],
                                    dwt[:, f * 128 : (f + 1) * 128],
                                    hts[f][:, co : co + 256],
                                    start=(f == 0),
                                    stop=(f == 10),
                                )
                            ot = pool_out.tile([128, 256], bf16, tag="otq", name="otq")
                            nc.vector.tensor_copy(ot[:], po[:])
                            nc.scalar.dma_start(
                                yt[hb * 128 : (hb + 1) * 128, co : co + 256], ot[:]
                            )
                        continue
                    po = ps.tile([128, 512], f32, tag="ps", name="po")
                    for f in range(11):
                        nc.tensor.matmul(
                            po[:],
                            dwt[:, f * 128 : (f + 1) * 128],
                            hts[f][:, cb * 512 : (cb + 1) * 512],
                            start=(f == 0),
                            stop=(f == 10),
                        )
                    ot = pool_out.tile([128, 512], bf16, tag="ot")
                    nc.vector.tensor_copy(ot[:], po[:])
                    nc.scalar.dma_start(
                        yt[hb * 128 : (hb + 1) * 128, cb * 512 : (cb + 1) * 512],
                        ot[:],
                    )
    nc.compile()
    return nc


def _get_compiled():
    global _compiled
    if _compiled is None:
        _compiled = _build()
    return _compiled


def _route(x, router_w):
    """fp64 router: returns per-expert (indices, gating weights)."""
    logits = x.astype(np.float64) @ router_w.astype(np.float64).T
    logits -= logits.max(axis=-1, keepdims=True)
    p = np.exp(logits)
    p /= p.sum(axis=-1, keepdims=True)
    top2 = np.argsort(-p, axis=-1)[:, :K]
    pv = np.take_along_axis(p, top2, axis=-1)
    wts = pv / (pv.sum(axis=-1, keepdims=True) + 1e-20)
    idxs, gws = [], []
    for e in range(E):
        tok, pos = np.nonzero(top2 == e)
        idxs.append(tok.astype(np.int64))
        gws.append(wts[tok, pos].astype(np.float32))
    return idxs, gws


def _tile_gu(wT):
    # gu[f_blk, k, hb*128+m] = wT[hb*128+k, f_blk*128+m]
    return (
        wT.reshape(16, 128, 11, 128)
        .transpose(2, 1, 0, 3)
        .reshape(11, 128, 2048)
        .copy()
    )


def _tile_dw(D):
    # dw[hb, k, f_blk*128+m] = D[hb*128+m, f_blk*128+k]
    return (
        D.reshape(16, 128, 11, 128).transpose(0, 3, 2, 1).reshape(16, 128, 1408).copy()
    )


def _swiglu_host(xg, gate, up, down):
    g = xg @ gate.T
    u = xg @ up.T
    h = (g / (1.0 + np.exp(-g))) * u
    return h @ down.T


def kernel(hidden_states, router_w, gate_w, up_w, down_w):
    import ml_dtypes
    from concourse import bass_utils

    bf16 = ml_dtypes.bfloat16
    x = np.ascontiguousarray(hidden_states.reshape(-1, H).astype(np.float32))
    idxs, gws = _route(x, router_w)

    in_maps = []
    spill = []  # (expert, token_indices, weights) handled on host
    for e in range(E):
        idx = idxs[e]
        if len(idx) > C:
            spill.append((e, idx[C:], gws[e][C:]))
            idx = idx[:C]
        xt = np.zeros((H, C), dtype=bf16)
        xt[:, : len(idx)] = x[idx].T.astype(bf16)
        gu = np.stack(
            [
                _tile_gu(gate_w[e].T.astype(np.float32)),
                _tile_gu(up_w[e].T.astype(np.float32)),
            ]
        ).astype(bf16)
        dw = _tile_dw(down_w[e].astype(np.float32)).astype(bf16)
        in_maps.append({"xt": xt, "gu": gu, "dw": dw})

    global _last_in_maps
    _last_in_maps = in_maps
    nc = _get_compiled()
    res = bass_utils.run_bass_kernel_spmd(
        nc, in_maps, core_ids=list(range(N_CORES))
    )

    out = np.zeros((T, H), dtype=np.float32)
    for e in range(E):
        # token indices are unique within one expert (a token's two experts
        # are distinct), so fancy-index += is an exact scatter-add
        idx = idxs[e][:C]
        w = gws[e][:C]
        y = res.results[e]["yt"].astype(np.float32)[:, : len(idx)].T
        out[idx] += w[:, None] * y
    for e, idx, w in spill:
        y = _swiglu_host(x[idx], gate_w[e], up_w[e], down_w[e]).astype(np.float32)
        out[idx] += w[:, None] * y
    return out.reshape(hidden_states.shape).astype(np.float32)


# revision 29
# speedup vs baseline: 1.2110x; 1.2110x over previous
"""MoE (top-2 of 8 experts, swiglu MLP) on 8 Trainium2 NeuronCores.

Strategy: expert parallelism — core e owns expert e's weights.
 - Host: router (fp64 softmax/top-2), gather each expert's tokens,
   pre-tile weights into the layouts the PE consumes directly, cast to
   bf16 (fp32 PSUM accumulation on device keeps the error ~3.5e-3 in
   the max-over-global-max metric).
 - Device (per core, SPMD one NEFF): single chunk of C=2048 columns
   (exactly T*K/8 — perfectly balanced), all matmul N-blocks 512.
   Stage A: hT = silu(gate_w.T @ xT) * (up_w.T @ xT); stage B:
   yT = down_w @ hT. Weights stream once (no re-chunking).
 - Host: combine — scale rows by gating weight and scatter-add into
   the full output. Tokens beyond the 2048-capacity of an expert are
   computed on the host (103 token-expert pairs for this input).

Shapes: T=8192 tokens, H=2048, F=1408, E=8, K=2, C=2048.
"""

import numpy as np

T, H, E, K, F = 8192, 2048, 8, 2, 1408
C = 2048  # token capacity per expert; overflow falls back to host
N_CORES = 8

_compiled = None


def _build():
    from contextlib import ExitStack

    import concourse.mybir as mybir
    import concourse.tile as tile
    from concourse import bacc

    f32 = mybir.dt.float32
    bf16 = mybir.dt.bfloat16

    nc = bacc.Bacc("TRN2", target_bir_lowering=False, debug=False, num_devices=N_CORES)
    xt = nc.dram_tensor("xt", [H, C], bf16, kind="ExternalInput").ap()
    gu = nc.dram_tensor("gu", [2, 11, 128, 2048], bf16, kind="ExternalInput").ap()
    dw = nc.dram_tensor("dw", [16, 128, 1408], bf16, kind="ExternalInput").ap()
    yt = nc.dram_tensor("yt", [H, C], bf16, kind="ExternalOutput").ap()

    with tile.TileContext(nc) as tc:
        with ExitStack() as ctx:
            pool_xt = ctx.enter_context(tc.tile_pool(name="xt", bufs=32))
            pool_gu = ctx.enter_context(tc.tile_pool(name="gu", bufs=3))
            pool_dw = ctx.enter_context(tc.tile_pool(name="dw", bufs=3))
            pool_h = ctx.enter_context(tc.tile_pool(name="h", bufs=11))
            pool_sil = ctx.enter_context(tc.tile_pool(name="sil", bufs=4))
            pool_out = ctx.enter_context(tc.tile_pool(name="out", bufs=4))
            pool_wrm = ctx.enter_context(tc.tile_pool(name="wrm", bufs=1))
            ps = ctx.enter_context(tc.tile_pool(name="ps", bufs=8, space="PSUM"))

            # PE warmup: the first ~16us are DMA-bound (startup loads), and
            # the PE's HAM clock gate only reaches 2.4 GHz after ~3.4us of
            # sustained matmul activity. Burn dummy matmuls on memset data
            # while the DMAs land so the real matmuls start at full clock.
            wrm_w = pool_wrm.tile([128, 128], bf16, tag="wrm_w")
            wrm_x = pool_wrm.tile([128, 512], bf16, tag="wrm_x")
            wrm_o = pool_wrm.tile([128, 512], f32, tag="wrm_o")
            nc.vector.memset(wrm_w[:], 0.0)
            nc.vector.memset(wrm_x[:], 0.0)
            wrm_p = ps.tile([128, 512], f32, tag="ps", name="wrm_p")
            N_WARM = 24
            for i in range(N_WARM):
                nc.tensor.matmul(
                    wrm_p[:], wrm_w[:], wrm_x[:],
                    start=(i == 0), stop=(i == N_WARM - 1),
                )
            nc.vector.tensor_copy(wrm_o[:], wrm_p[:])

            # f=0..2 weights on the gpsimd (SWDGE) ring: they land during
            # the startup window without occupying the two HWDGE rings,
            # which stream the xt tiles at full rate right behind them
            guts, uuts = {}, {}
            for f in range(3):
                guts[f] = pool_gu.tile([128, 2048], bf16, tag="gut", name="gut")
                uuts[f] = pool_gu.tile([128, 2048], bf16, tag="uut", name="uut")
                nc.gpsimd.dma_start(guts[f][:], gu[0, f])
                nc.gpsimd.dma_start(uuts[f][:], gu[1, f])

            # token activations, H on partitions. Half-tiles (cols 0:1024
            # first, then 1024:2048) alternating across the two HWDGE rings:
            # stage A's first pass consumes cols 0:1024 of each h-tile at
            # ~850ns each, matching the ~700ns DMA delivery rate.
            xts = [[None] * 16, [None] * 16]  # [half][hb] -> [128, 1024] tile
            for half in range(2):
                for hb in range(16):
                    xtile = pool_xt.tile([128, C // 2], bf16, tag="xtile", name="xt")
                    xts[half][hb] = xtile
                    eng = nc.sync if hb % 2 == 0 else nc.scalar
                    eng.dma_start(
                        xtile[:],
                        xt[hb * 128 : (hb + 1) * 128, half * 1024 : (half + 1) * 1024],
                    )

            def silu_mul(pg, pu, ht, gcb):
                sil = pool_sil.tile([128, 512], f32, tag="sil", name="sil")
                nc.scalar.activation(sil[:], pg[:], mybir.ActivationFunctionType.Silu)
                nc.vector.tensor_mul(ht[:, gcb * 512 : (gcb + 1) * 512], sil[:], pu[:])

            # stage A: hT[f, c] = silu(gT) * uT, gT = gate_w.T @ x.T
            # g and u interleaved per h-tile in column-block pairs, so each
            # xt half-tile is fully consumed as soon as it lands.
            hts = []
            for f in range(11):
                if f in guts:
                    gut, uut = guts[f], uuts[f]
                else:
                    gut = pool_gu.tile([128, 2048], bf16, tag="gut", name="gut")
                    uut = pool_gu.tile([128, 2048], bf16, tag="uut", name="uut")
                    nc.sync.dma_start(gut[:], gu[0, f])
                    nc.scalar.dma_start(uut[:], gu[1, f])
                ht = pool_h.tile([128, C], bf16, tag="ht")
                hts.append(ht)
                for part in range(2):
                    cbs = (0, 1)  # column blocks within this half's tiles
                    pgs = [ps.tile([128, 512], f32, tag="ps", name="pg") for _ in cbs]
                    pus = [ps.tile([128, 512], f32, tag="ps", name="pu") for _ in cbs]
                    for h in range(16):
                        for cb, pg in zip(cbs, pgs):
                            nc.tensor.matmul(
                                pg[:],
                                gut[:, h * 128 : (h + 1) * 128],
                                xts[part][h][:, cb * 512 : (cb + 1) * 512],
                                start=(h == 0),
                                stop=(h == 15),
                            )
                        for cb, pu in zip(cbs, pus):
                            nc.tensor.matmul(
                                pu[:],
                                uut[:, h * 128 : (h + 1) * 128],
                                xts[part][h][:, cb * 512 : (cb + 1) * 512],
                                start=(h == 0),
                                stop=(h == 15),
                            )
                    for cb, pg, pu in zip(cbs, pgs, pus):
                        silu_mul(pg, pu, ht, 2 * part + cb)

            # stage B: yT[h, c] = down_w @ hT  (gating applied on host).
            # Output stored bf16 (upcast on host) — halves store traffic.
            for hb in range(16):
                dwt = pool_dw.tile([128, 1408], bf16, tag="dwt")
                nc.sync.dma_start(dwt[:], dw[hb])
                for cb in range(4):
                    if hb == 15 and cb == 3:
                        # final group: two 256-wide accumulations so the
                        # tail-gating cast+store chain covers 64KB, not 128KB
                        for q in range(2):
                            co = cb * 512 + q * 256
                            po = ps.tile([128, 256], f32, tag="ps", name="po")
                            for f in range(11):
                                nc.tensor.matmul(
                                    po[:],
                                    dwt[:, f * 128 : (f + 1) * 128],
                                    hts[f][:, co : co + 256],
                                    start=(f == 0),
                                    stop=(f == 10),
                                )
                            ot = pool_out.tile([128, 256], bf16, tag="otq", name="otq")
                            nc.vector.tensor_copy(ot[:], po[:])
                            nc.scalar.dma_start(
                                yt[hb * 128 : (hb + 1) * 128, co : co + 256], ot[:]
                            )
                        continue
                    po = ps.tile([128, 512], f32, tag="ps", name="po")
                    for f in range(11):
                        nc.tensor.matmul(
                            po[:],
                            dwt[:, f * 128 : (f + 1) * 128],
                            hts[f][:, cb * 512 : (cb + 1) * 512],
                            start=(f == 0),
                            stop=(f == 10),
                        )
                    ot = pool_out.tile([128, 512], bf16, tag="ot")
                    nc.vector.tensor_copy(ot[:], po[:])
                    nc.scalar.dma_start(
                        yt[hb * 128 : (hb + 1) * 128, cb * 512 : (cb + 1) * 512],
                        ot[:],
                    )
    nc.compile()
    return nc


def _get_compiled():
    global _compiled
    if _compiled is None:
        _compiled = _build()
    return _compiled


def _route(x, router_w):
    """fp64 router: returns per-expert (indices, gating weights)."""
    logits = x.astype(np.float64) @ router_w.astype(np.float64).T
    logits -= logits.max(axis=-1, keepdims=True)
    p = np.exp(logits)
    p /= p.sum(axis=-1, keepdims=True)
    top2 = np.argsort(-p, axis=-1)[:, :K]
    pv = np.take_along_axis(p, top2, axis=-1)
    wts = pv / (pv.sum(axis=-1, keepdims=True) + 1e-20)
    idxs, gws = [], []
    for e in range(E):
        tok, pos = np.nonzero(top2 == e)
        idxs.append(tok.astype(np.int64))
        gws.append(wts[tok, pos].astype(np.float32))
    return idxs, gws


def _tile_gu(wT):
    # gu[f_blk, k, hb*128+m] = wT[hb*128+k, f_blk*128+m]
    return (
        wT.reshape(16, 128, 11, 128)
        .transpose(2, 1, 0, 3)
        .reshape(11, 128, 2048)
        .copy()
    )


def _tile_dw(D):
    # dw[hb, k, f_blk*128+m] = D[hb*128+m, f_blk*128+k]
    return (
        D.reshape(16, 128, 11, 128).transpose(0, 3, 2, 1).reshape(16, 128, 1408).copy()
    )


def _swiglu_host(xg, gate, up, down):
    g = xg @ gate.T
    u = xg @ up.T
    h = (g / (1.0 + np.exp(-g))) * u
    return h @ down.T


def kernel(hidden_states, router_w, gate_w, up_w, down_w):
    import ml_dtypes
    from concourse import bass_utils

    bf16 = ml_dtypes.bfloat16
    x = np.ascontiguousarray(hidden_states.reshape(-1, H).astype(np.float32))
    idxs, gws = _route(x, router_w)

    in_maps = []
    spill = []  # (expert, token_indices, weights) handled on host
    for e in range(E):
        idx = idxs[e]
        if len(idx) > C:
            spill.append((e, idx[C:], gws[e][C:]))
            idx = idx[:C]
        xt = np.zeros((H, C), dtype=bf16)
        xt[:, : len(idx)] = x[idx].T.astype(bf16)
        gu = np.stack(
            [
                _tile_gu(gate_w[e].T.astype(np.float32)),
                _tile_gu(up_w[e].T.astype(np.float32)),
            ]
        ).astype(bf16)
        dw = _tile_dw(down_w[e].astype(np.float32)).astype(bf16)
        in_maps.append({"xt": xt, "gu": gu, "dw": dw})

    global _last_in_maps
    _last_in_maps = in_maps
    nc = _get_compiled()
    res = bass_utils.run_bass_kernel_spmd(
        nc, in_maps, core_ids=list(range(N_CORES))
    )

    out = np.zeros((T, H), dtype=np.float32)
    for e in range(E):
        # token indices are unique within one expert (a token's two experts
        # are distinct), so fancy-index += is an exact scatter-add
        idx = idxs[e][:C]
        w = gws[e][:C]
        y = res.results[e]["yt"].astype(np.float32)[:, : len(idx)].T
        out[idx] += w[:, None] * y
    for e, idx, w in spill:
        y = _swiglu_host(x[idx], gate_w[e], up_w[e], down_w[e]).astype(np.float32)
        out[idx] += w[:, None] * y
    return out.reshape(hidden_states.shape).astype(np.float32)
